# revision 13
# baseline (speedup 1.0000x reference)
"""Trainium2 Bass kernel for nn_MultiHeadGlobalMemory.

Contract: kernel(**inputs) takes FULL unsharded numpy inputs and returns the
FULL outputs (enhanced (N, DIM) fp32, beta_avg (N,) fp32), matching
reference.reference(). Shards entity rows over 8 NeuronCores; two-phase
kernel with one small AllReduce between phases.

Phase A (softmax stats, bf16): per 512-row block, e^T = Ablk^T @ X^T on PE,
p = max(exp(z), exp(0.2 z)) (== exp(leaky_relu(z))) on ACT/DVE, transpose p
to natural layout, then pool += p^T @ X and sumexp += p^T @ 1 accumulate in
PSUM across all blocks.  AllReduce(pool, sumexp) over the 8 cores.

Phase B (fp32): g = pool/sumexp, c1 = W1b^T g + b1, Gw[h,:] = per-head
g-weighted rows of Wout (+ bout row).  Per block: hidT = relu(W1a^T X^T +
c1), betaT = sigmoid(W2^T hidT + b2), bcast (1-beta) over head partitions
via sel-matmul, enh1T = X^T * (1-B)^T, out = enh1^T^T @ Wout + beta_ext @
Gw_ext (fused bias), beta_avg = mean(beta).

Softmax runs without max-subtraction (e is O(5), exp safe in fp32); the
pool/sumexp ratio is mathematically identical to the reference softmax.
"""

import sys
import types

import numpy as np

# ---------------------------------------------------------------- axon hook
def _install_ntff_hook():
    """Provide antenv.axon_hooks so trace=True works under axon (optional)."""
    if "antenv.axon_hooks" in sys.modules:
        return
    try:
        state = {"hook": None}
        mod = types.ModuleType("antenv.axon_hooks")
        mod.set_axon_ntff_profile_hook = lambda h: state.__setitem__("hook", h)
        mod.get_axon_ntff_profile_hook = lambda: state["hook"]
        sys.modules["antenv.axon_hooks"] = mod
        from trn_agent_boot.trn_boot import _ntff_profile_via_ctypes

        mod.set_axon_ntff_profile_hook(
            _ntff_profile_via_ctypes("/opt/axon/libaxon_pjrt.so")
        )
    except Exception:
        pass


_install_ntff_hook()

import ml_dtypes  # noqa: E402
import concourse.bacc as bacc  # noqa: E402
import concourse.tile as tile  # noqa: E402
from concourse import mybir  # noqa: E402
from concourse.bass_utils import run_bass_kernel_spmd  # noqa: E402

F32 = mybir.dt.float32
F32R = mybir.dt.float32r
BF16 = mybir.dt.bfloat16
AF = mybir.ActivationFunctionType
ALU = mybir.AluOpType

N_CORES = 8
DIM, H, HD = 512, 8, 64
BLK = 512
TILE = 128
ASUPER = 4          # phase-A blocks per DMA superblock

_cache = {}


def _build(nblk: int, valid_rows: int):
    RP = nblk * BLK
    nc = bacc.Bacc("TRN2", target_bir_lowering=False, num_devices=N_CORES)

    ns = (nblk + ASUPER - 1) // ASUPER
    xbf = nc.dram_tensor("xbf", [128, ns, 4 * ASUPER, DIM], BF16, kind="ExternalInput")
    xtbf = nc.dram_tensor("xtbf", [128, ns, 4, ASUPER * BLK], BF16, kind="ExternalInput")
    xt = nc.dram_tensor("xt", [128, nblk, 4, BLK], F32R, kind="ExternalInput")
    ablk = nc.dram_tensor("ablk", [DIM, H], BF16, kind="ExternalInput")
    ch1 = nc.dram_tensor("ch1", [H, 1], F32, kind="ExternalInput")
    ch2 = nc.dram_tensor("ch2", [H, 1], F32, kind="ExternalInput")
    w1ad = nc.dram_tensor("w1ad", [DIM, 128], F32R, kind="ExternalInput")
    w1bd = nc.dram_tensor("w1bd", [DIM, 128], F32, kind="ExternalInput")
    b1t = nc.dram_tensor("b1t", [DIM, 1], F32, kind="ExternalInput")
    w2blk = nc.dram_tensor("w2blk", [DIM, H], F32R, kind="ExternalInput")
    b2c = nc.dram_tensor("b2c", [H, 1], F32, kind="ExternalInput")
    wout = nc.dram_tensor("wout", [DIM, DIM], F32, kind="ExternalInput")
    woutr = nc.dram_tensor("woutr", [DIM, DIM], F32R, kind="ExternalInput")
    seld = nc.dram_tensor("seld", [H, DIM], F32R, kind="ExternalInput")
    mask8d = nc.dram_tensor("mask8d", [DIM, H], F32, kind="ExternalInput")
    id8 = nc.dram_tensor("id8", [H, H], F32, kind="ExternalInput")
    id8b = nc.dram_tensor("id8b", [H, H], BF16, kind="ExternalInput")

    enh = nc.dram_tensor("enh", [RP, DIM], F32, kind="ExternalOutput")
    bavg = nc.dram_tensor("bavg", [1, RP], F32, kind="ExternalOutput")

    # columns of the last block holding padded (invalid) rows
    pad_from = valid_rows - (nblk - 1) * BLK
    assert 0 < pad_from <= BLK

    with tile.TileContext(nc) as tc:
        with tc.tile_pool(name="consts", bufs=1) as cp, \
             tc.tile_pool(name="dram", bufs=1, space="DRAM") as dramp:
            ablkt = cp.tile([128, 4 * H], BF16)
            nc.sync.dma_start(
                ablkt[:].rearrange("p (j h) -> p j h", h=H),
                ablk[:].rearrange("(j p) h -> p j h", p=128),
            )
            ch1t = cp.tile([H, 1], F32)
            nc.sync.dma_start(ch1t[:], ch1[:])
            ch2t = cp.tile([H, 1], F32)
            nc.sync.dma_start(ch2t[:], ch2[:])
            ones_bf = cp.tile([TILE, 1], BF16)
            nc.vector.memset(ones_bf[:], 1.0)
            ones8 = cp.tile([H, 1], F32R)
            nc.vector.memset(ones8[:].bitcast(F32), 1.0)

            w1adt = cp.tile([128, 4 * 128], F32R)
            nc.sync.dma_start(
                w1adt[:].rearrange("p (j e) -> p j e", e=128),
                w1ad[:].rearrange("(j p) e -> p j e", p=128),
            )
            w1bdt = cp.tile([128, 4 * 128], F32)
            nc.sync.dma_start(
                w1bdt[:].rearrange("p (j e) -> p j e", e=128),
                w1bd[:].rearrange("(j p) e -> p j e", p=128),
            )
            b1tt = cp.tile([128, 4], F32)
            nc.sync.dma_start(
                b1tt[:], b1t[:].rearrange("(j p) o -> p (j o)", p=128)
            )
            w2t = cp.tile([128, 4 * H], F32R)
            nc.sync.dma_start(
                w2t[:].rearrange("p (j h) -> p j h", h=H),
                w2blk[:].rearrange("(j p) h -> p j h", p=128),
            )
            b2t = cp.tile([H, 1], F32)
            nc.sync.dma_start(b2t[:], b2c[:])
            woutt = cp.tile([128, 4 * DIM], F32)
            nc.sync.dma_start(
                woutt[:].rearrange("p (j m) -> p j m", m=DIM),
                wout[:].rearrange("(j p) m -> p j m", p=128),
            )
            selt = cp.tile([H, DIM], F32R)
            nc.sync.dma_start(selt[:], seld[:])
            woutrt = cp.tile([128, 4 * DIM], F32R)
            nc.sync.dma_start(
                woutrt[:].rearrange("p (j m) -> p j m", m=DIM),
                woutr[:].rearrange("(j p) m -> p j m", p=128),
            )
            mask8t = cp.tile([128, 4 * H], F32)
            nc.sync.dma_start(
                mask8t[:].rearrange("p (j h) -> p j h", h=H),
                mask8d[:].rearrange("(j p) h -> p j h", p=128),
            )
            id8t = cp.tile([H, H], F32)
            nc.sync.dma_start(id8t[:], id8[:])
            id8bt = cp.tile([H, H], BF16)
            nc.sync.dma_start(id8bt[:], id8b[:])

            gwext = cp.tile([H, DIM], F32R)
            c1s = cp.tile([128, 4], F32)
            gtcol = cp.tile([128, 4], F32)
            bavgs = cp.tile([1, RP], F32)

            # ================================================== PHASE A ==
            with tc.tile_pool(name="pa_x", bufs=2) as pax, \
                 tc.tile_pool(name="pa_s", bufs=4) as pas, \
                 tc.tile_pool(name="pa_eps", bufs=2, space="PSUM") as paeps, \
                 tc.tile_pool(name="pa_tps", bufs=2, space="PSUM") as patps, \
                 tc.tile_pool(name="pa_acc", bufs=1, space="PSUM") as paacc:
                poolacc = paacc.tile([H, DIM], F32)
                seacc = paacc.tile([H, 1], F32)
                for s0 in range(0, nblk, ASUPER):
                    cnt = min(ASUPER, nblk - s0)
                    ncols = cnt * BLK
                    s = s0 // ASUPER
                    xn = pax.tile([TILE, ASUPER * 4 * DIM], BF16, tag="xn")
                    nc.sync.dma_start(
                        xn[:, : cnt * 4 * DIM].rearrange(
                            "p (t d) -> p t d", d=DIM
                        ),
                        xbf[:, s, 0 : cnt * 4, :],
                    )
                    xtn = pax.tile([TILE, ASUPER * 4 * BLK], BF16, tag="xtn")
                    nc.sync.dma_start(
                        xtn[:, : cnt * 4 * BLK].rearrange(
                            "p (j n) -> p j n", n=ncols
                        ),
                        xtbf[:, s, :, 0 : cnt * BLK],
                    )
                    for bi in range(cnt):
                        b = s0 + bi
                        et_ps = paeps.tile([H, BLK], F32, tag="etps")
                        for j in range(4):
                            nc.tensor.matmul(
                                et_ps[:],
                                ablkt[:, j * H : (j + 1) * H],
                                xtn[:, j * ncols + bi * BLK :
                                       j * ncols + (bi + 1) * BLK],
                                start=(j == 0), stop=(j == 3),
                            )
                        p1 = pas.tile([H, BLK], BF16, tag="p1")
                        nc.scalar.activation(
                            p1[:], et_ps[:], AF.Exp, bias=ch1t[:], scale=1.0
                        )
                        p2 = pas.tile([H, BLK], BF16, tag="p2")
                        nc.scalar.activation(
                            p2[:], et_ps[:], AF.Exp, bias=ch2t[:], scale=0.2
                        )
                        nc.vector.tensor_tensor(p1[:], p1[:], p2[:], ALU.max)
                        if b == nblk - 1 and pad_from < BLK:
                            nc.vector.memset(
                                p1[:, pad_from:BLK].bitcast(F32), 0.0
                            )
                        first, last = b == 0, b == nblk - 1
                        for t in range(4):
                            tr_ps = patps.tile([128, H], BF16, tag="trps")
                            nc.tensor.transpose(
                                tr_ps[:],
                                p1[:, t * 128 : (t + 1) * 128],
                                id8bt[:],
                            )
                            pn = pas.tile([128, H], BF16, tag="pn")
                            nc.scalar.copy(pn[:], tr_ps[:])
                            xslice = xn[:, (bi * 4 + t) * DIM :
                                           (bi * 4 + t + 1) * DIM]
                            nc.tensor.matmul(
                                poolacc[:], pn[:], xslice,
                                start=(first and t == 0),
                                stop=(last and t == 3),
                                skip_group_check=True,
                            )
                            nc.tensor.matmul(
                                seacc[:], pn[:], ones_bf[:],
                                start=(first and t == 0),
                                stop=(last and t == 3),
                                skip_group_check=True,
                            )

                pool_s = cp.tile([H, DIM], F32)
                nc.scalar.copy(pool_s[:], poolacc[:])
                se_s = cp.tile([H, 1], F32)
                nc.scalar.copy(se_s[:], seacc[:])

            # ============================================== COLLECTIVE ==
            cc_in = dramp.tile([H, DIM + 1], F32)
            cc_out = dramp.tile([H, DIM + 1], F32)
            nc.sync.dma_start(cc_in[:, 0:DIM], pool_s[:])
            nc.sync.dma_start(cc_in[:, DIM : DIM + 1], se_s[:])
            nc.gpsimd.collective_compute(
                "AllReduce",
                ALU.add,
                replica_groups=[list(range(N_CORES))],
                ins=[cc_in.opt()],
                outs=[cc_out.opt()],
            )
            with tc.tile_pool(name="g_ps", bufs=2, space="PSUM") as gps:
                gsum = cp.tile([H, DIM + 1], F32)
                nc.sync.dma_start(gsum[:], cc_out[:])
                recip = cp.tile([H, 1], F32)
                nc.vector.reciprocal(recip[:], gsum[:, DIM : DIM + 1])
                g8 = cp.tile([H, DIM], F32)
                nc.vector.tensor_scalar(
                    g8[:], gsum[:, 0:DIM], recip[:], None, ALU.mult
                )
                gw_ps = gps.tile([H, DIM], F32, tag="gw")
                for j in range(4):
                    tr_ps = gps.tile([128, H], F32, tag="tr")
                    nc.tensor.transpose(
                        tr_ps[:], g8[:, j * 128 : (j + 1) * 128], id8t[:]
                    )
                    gt_j = cp.tile([128, H], F32, tag=f"gt{j}")
                    nc.scalar.copy(gt_j[:], tr_ps[:])
                    msk_j = mask8t[:, j * H : (j + 1) * H]
                    nc.vector.tensor_tensor(gt_j[:], gt_j[:], msk_j, ALU.mult)
                    nc.vector.tensor_reduce(
                        gtcol[:, j : j + 1], gt_j[:],
                        mybir.AxisListType.X, ALU.add,
                    )
                    gsel_j = cp.tile([128, H], F32, tag=f"gsel{j}")
                    nc.vector.tensor_scalar(
                        gsel_j[:], msk_j, gtcol[:, j : j + 1], None, ALU.mult
                    )
                    nc.tensor.matmul(
                        gw_ps[:], gsel_j[:],
                        woutt[:, j * DIM : (j + 1) * DIM],
                        start=(j == 0), stop=(j == 3),
                    )
                    c1_ps = gps.tile([128, 1], F32, tag="c1ps")
                    nc.tensor.matmul(
                        c1_ps[:], w1bdt[:, j * 128 : (j + 1) * 128],
                        gtcol[:, j : j + 1], start=True, stop=True,
                    )
                    nc.scalar.activation(
                        c1s[:, j : j + 1], c1_ps[:], AF.Identity,
                        bias=b1tt[:, j : j + 1],
                    )
                nc.scalar.copy(gwext[0:H, :], gw_ps[:])

            # ================================================== PHASE B ==
            with tc.tile_pool(name="pb_x", bufs=3) as pbx, \
                 tc.tile_pool(name="pb_w", bufs=3) as pbw, \
                 tc.tile_pool(name="pb_o", bufs=2) as pbo, \
                 tc.tile_pool(name="pb_hps", bufs=3, space="PSUM") as pbhps, \
                 tc.tile_pool(name="pb_bps", bufs=1, space="PSUM") as pbbps, \
                 tc.tile_pool(name="pb_cps", bufs=1, space="PSUM") as pbcps, \
                 tc.tile_pool(name="pb_ops", bufs=2, space="PSUM") as pbops:
                for b in range(nblk):
                    n0 = b * BLK
                    xtr = pbx.tile([128, 4 * BLK], F32R, tag="xtr")
                    nc.sync.dma_start(
                        xtr[:].rearrange("p (j n) -> p j n", n=BLK),
                        xt[:, b, :, :],
                    )
                    beta_ps = pbbps.tile([H, BLK], F32, tag="betaps")
                    for j in range(4):
                        hid_ps = pbhps.tile([128, BLK], F32, tag="hidps")
                        nc.tensor.matmul(
                            hid_ps[:], w1adt[:, j * 128 : (j + 1) * 128],
                            xtr[:, j * BLK : (j + 1) * BLK],
                            start=True, stop=True,
                        )
                        hid_s = pbw.tile([128, BLK], F32R, tag="hids")
                        nc.scalar.activation(
                            hid_s[:], hid_ps[:], AF.Relu,
                            bias=c1s[:, j : j + 1],
                        )
                        nc.tensor.matmul(
                            beta_ps[:], w2t[:, j * H : (j + 1) * H], hid_s[:],
                            start=(j == 0), stop=(j == 3),
                        )
                    betax = pbw.tile([H, BLK], F32R, tag="betax")
                    nc.scalar.activation(
                        betax[0:H, :], beta_ps[:], AF.Sigmoid, bias=b2t[:]
                    )
                    beta1 = pbw.tile([H, BLK], F32R, tag="beta1")
                    nc.vector.tensor_scalar(
                        beta1[:], betax[0:H, :].bitcast(F32), -1.0, 1.0,
                        ALU.mult, ALU.add,
                    )
                    enh1 = pbo.tile([128, 4 * BLK], F32R, tag="enh1")
                    for j in range(4):
                        bc_ps = pbcps.tile([128, BLK], F32, tag="bcps")
                        nc.tensor.matmul(
                            bc_ps[:], selt[:, j * 128 : (j + 1) * 128],
                            beta1[:], start=True, stop=True,
                        )
                        nc.vector.tensor_tensor(
                            enh1[:, j * BLK : (j + 1) * BLK],
                            xtr[:, j * BLK : (j + 1) * BLK].bitcast(F32),
                            bc_ps[:], ALU.mult,
                        )
                    outb = pbo.tile([128, 4 * DIM], F32, tag="outb")
                    for t in range(4):
                        out_ps = pbops.tile([128, DIM], F32, tag="outps")
                        for j in range(4):
                            nc.tensor.matmul(
                                out_ps[:],
                                enh1[:, j * BLK + t * 128 :
                                        j * BLK + (t + 1) * 128],
                                woutrt[:, j * DIM : (j + 1) * DIM],
                                start=(j == 0), stop=False,
                                skip_group_check=True,
                            )
                        nc.tensor.matmul(
                            out_ps[:], betax[:, t * 128 : (t + 1) * 128],
                            gwext[:], start=False, stop=True,
                            skip_group_check=True,
                        )
                        if t % 2 == 0:
                            nc.vector.tensor_copy(
                                outb[:, t * DIM : (t + 1) * DIM], out_ps[:]
                            )
                        else:
                            nc.scalar.copy(
                                outb[:, t * DIM : (t + 1) * DIM], out_ps[:]
                            )
                    avg_ps = pbbps.tile([1, BLK], F32, tag="avgps")
                    nc.tensor.matmul(
                        avg_ps[:], ones8[:], betax[0:H, :],
                        start=True, stop=True,
                    )
                    nc.scalar.activation(
                        bavgs[:, n0 : n0 + BLK], avg_ps[:], AF.Copy,
                        scale=0.125,
                    )
                    nc.sync.dma_start(
                        enh[n0 : n0 + BLK, :].rearrange(
                            "(t p) d -> p t d", p=128
                        ),
                        outb[:].rearrange("p (t d) -> p t d", d=DIM),
                    )
                nc.sync.dma_start(bavg[:], bavgs[:])
    nc.compile()
    return nc


def _preprocess(entity_embeds, global_memories, attn_vec, W1, b1, W2, b2,
                Wout, bout, nblk, rows_valid):
    RP = nblk * BLK
    ent = np.ascontiguousarray(entity_embeds, dtype=np.float32)
    n = ent.shape[0]
    assert n == N_CORES * rows_valid

    a_g = attn_vec[:, :HD].astype(np.float32)
    a_h = attn_vec[:, HD:].astype(np.float32)
    ablk = np.zeros((DIM, H), np.float32)
    for h in range(H):
        ablk[h * HD : (h + 1) * HD, h] = a_h[h]
    ch = (global_memories.astype(np.float32) * a_g).sum(axis=1).reshape(H, 1)

    W1f = np.asarray(W1, np.float32)
    w1ad = np.zeros((DIM, 128), np.float32)
    w1bd = np.zeros((DIM, 128), np.float32)
    for h in range(H):
        r0 = h * HD
        c0 = (h * HD) % 128
        w1ad[r0 : r0 + HD, c0 : c0 + HD] = W1f[h, :HD, :]
        w1bd[r0 : r0 + HD, c0 : c0 + HD] = W1f[h, HD:, :]
    b1t = np.asarray(b1, np.float32).reshape(DIM, 1)
    w2blk = np.zeros((DIM, H), np.float32)
    for h in range(H):
        w2blk[h * HD : (h + 1) * HD, h] = np.asarray(W2[h], np.float32)
    b2c = np.asarray(b2, np.float32).reshape(H, 1)

    heads = np.arange(DIM) // HD
    sel = (heads[None, :] == np.arange(H)[:, None]).astype(np.float32)
    mask8 = sel.T.copy()
    id8 = np.eye(H, dtype=np.float32)

    common = {
        "ablk": ablk.astype(ml_dtypes.bfloat16),
        "ch1": ch,
        "ch2": 0.2 * ch,
        "w1ad": w1ad,
        "w1bd": w1bd,
        "b1t": b1t,
        "w2blk": w2blk,
        "b2c": b2c,
        "wout": np.asarray(Wout, np.float32),
        "woutr": np.asarray(Wout, np.float32),
        "seld": np.ascontiguousarray(sel),
        "mask8d": np.ascontiguousarray(mask8),
        "id8": id8,
        "id8b": id8.astype(ml_dtypes.bfloat16),
    }
    ns = (nblk + ASUPER - 1) // ASUPER
    RP2 = ns * ASUPER * BLK
    in_maps = []
    for c in range(N_CORES):
        sl = ent[c * rows_valid : (c + 1) * rows_valid]
        pad = np.zeros((RP2, DIM), np.float32)
        pad[:rows_valid] = sl
        padb = pad.astype(ml_dtypes.bfloat16)
        # natural swizzle: [p, s, t, d] = pad[s*2048 + t*128 + p, d]
        xbf_sw = np.ascontiguousarray(
            padb.reshape(ns, 4 * ASUPER, 128, DIM).transpose(2, 0, 1, 3)
        )
        # transposed swizzles from pad.T (DIM, RP2)
        xtb = np.ascontiguousarray(pad.T).astype(ml_dtypes.bfloat16)
        xtbf_sw = np.ascontiguousarray(
            xtb.reshape(4, 128, ns, ASUPER * BLK).transpose(1, 2, 0, 3)
        )
        xtf = np.ascontiguousarray(pad[:RP].T)  # (DIM, RP) fp32
        xt_sw = np.ascontiguousarray(
            xtf.reshape(4, 128, nblk, BLK).transpose(1, 2, 0, 3)
        )
        m = dict(common)
        m["xbf"] = xbf_sw
        m["xtbf"] = xtbf_sw
        m["xt"] = xt_sw
        in_maps.append(m)
    return in_maps


def run(entity_embeds, global_memories, attn_vec, W1, b1, W2, b2, Wout, bout,
        nblk=None, trace=False):
    n = entity_embeds.shape[0]
    rows = n // N_CORES
    if nblk is None:
        nblk = (rows + BLK - 1) // BLK
    key = (nblk, rows)
    if key not in _cache:
        _cache[key] = _build(nblk, rows)
    nc = _cache[key]
    in_maps = _preprocess(
        entity_embeds, global_memories, attn_vec, W1, b1, W2, b2, Wout, bout,
        nblk, rows,
    )
    res = run_bass_kernel_spmd(
        nc, in_maps, core_ids=list(range(N_CORES)), trace=trace
    )
    enh = np.concatenate(
        [res.results[c]["enh"][:rows] for c in range(N_CORES)], axis=0
    )
    enh += np.asarray(bout, np.float32)[None, :]
    bavg = np.concatenate(
        [res.results[c]["bavg"][0, :rows] for c in range(N_CORES)], axis=0
    )
    return (enh, bavg), res


def kernel(entity_embeds, global_memories, attn_vec, W1, b1, W2, b2, Wout,
           bout):
    (enh, bavg), _ = run(
        entity_embeds, global_memories, attn_vec, W1, b1, W2, b2, Wout, bout
    )
    return enh, bavg


# revision 14
# speedup vs baseline: 1.0538x; 1.0538x over previous
"""Trainium2 Bass kernel for nn_MultiHeadGlobalMemory.

Contract: kernel(**inputs) takes FULL unsharded numpy inputs and returns the
FULL outputs (enhanced (N, DIM) fp32, beta_avg (N,) fp32), matching
reference.reference(). Shards entity rows over 8 NeuronCores; two-phase
kernel with one small AllReduce between phases.

Phase A (softmax stats, bf16): per 512-row block, e^T = Ablk^T @ X^T on PE,
p = max(exp(z), exp(0.2 z)) (== exp(leaky_relu(z))) on ACT/DVE, transpose p
to natural layout, then pool += p^T @ X and sumexp += p^T @ 1 accumulate in
PSUM across all blocks.  AllReduce(pool, sumexp) over the 8 cores.

Phase B (fp32): g = pool/sumexp, c1 = W1b^T g + b1, Gw[h,:] = per-head
g-weighted rows of Wout (+ bout row).  Per block: hidT = relu(W1a^T X^T +
c1), betaT = sigmoid(W2^T hidT + b2), bcast (1-beta) over head partitions
via sel-matmul, enh1T = X^T * (1-B)^T, out = enh1^T^T @ Wout + beta_ext @
Gw_ext (fused bias), beta_avg = mean(beta).

Softmax runs without max-subtraction (e is O(5), exp safe in fp32); the
pool/sumexp ratio is mathematically identical to the reference softmax.
"""

import sys
import types

import numpy as np

# ---------------------------------------------------------------- axon hook
def _install_ntff_hook():
    """Provide antenv.axon_hooks so trace=True works under axon (optional)."""
    if "antenv.axon_hooks" in sys.modules:
        return
    try:
        state = {"hook": None}
        mod = types.ModuleType("antenv.axon_hooks")
        mod.set_axon_ntff_profile_hook = lambda h: state.__setitem__("hook", h)
        mod.get_axon_ntff_profile_hook = lambda: state["hook"]
        sys.modules["antenv.axon_hooks"] = mod
        from trn_agent_boot.trn_boot import _ntff_profile_via_ctypes

        mod.set_axon_ntff_profile_hook(
            _ntff_profile_via_ctypes("/opt/axon/libaxon_pjrt.so")
        )
    except Exception:
        pass


_install_ntff_hook()

import ml_dtypes  # noqa: E402
import concourse.bacc as bacc  # noqa: E402
import concourse.tile as tile  # noqa: E402
from concourse import mybir  # noqa: E402
from concourse.bass_utils import run_bass_kernel_spmd  # noqa: E402

F32 = mybir.dt.float32
F32R = mybir.dt.float32r
BF16 = mybir.dt.bfloat16
AF = mybir.ActivationFunctionType
ALU = mybir.AluOpType

N_CORES = 8
DIM, H, HD = 512, 8, 64
BLK = 512
TILE = 128
ASUPER = 4          # phase-A blocks per DMA superblock

_cache = {}


def _build(nblk: int, valid_rows: int):
    RP = nblk * BLK
    nc = bacc.Bacc("TRN2", target_bir_lowering=False, num_devices=N_CORES)

    ns = (nblk + ASUPER - 1) // ASUPER
    xbf = nc.dram_tensor("xbf", [128, ns, 4 * ASUPER, DIM], BF16, kind="ExternalInput")
    xtbf = nc.dram_tensor("xtbf", [128, ns, 4, ASUPER * BLK], BF16, kind="ExternalInput")
    xt = nc.dram_tensor("xt", [128, nblk, 4, BLK], F32R, kind="ExternalInput")
    ablk = nc.dram_tensor("ablk", [DIM, H], BF16, kind="ExternalInput")
    ch1 = nc.dram_tensor("ch1", [H, 1], F32, kind="ExternalInput")
    c08 = nc.dram_tensor("c08", [H, 1], F32, kind="ExternalInput")
    w1ad = nc.dram_tensor("w1ad", [DIM, 128], F32R, kind="ExternalInput")
    w1bd = nc.dram_tensor("w1bd", [DIM, 128], F32, kind="ExternalInput")
    b1t = nc.dram_tensor("b1t", [DIM, 1], F32, kind="ExternalInput")
    w2blk = nc.dram_tensor("w2blk", [DIM, H], F32R, kind="ExternalInput")
    b2c = nc.dram_tensor("b2c", [H, 1], F32, kind="ExternalInput")
    wout = nc.dram_tensor("wout", [DIM, DIM], F32, kind="ExternalInput")
    woutr = nc.dram_tensor("woutr", [DIM, DIM], F32R, kind="ExternalInput")
    seld = nc.dram_tensor("seld", [H, DIM], F32R, kind="ExternalInput")
    mask8d = nc.dram_tensor("mask8d", [DIM, H], F32, kind="ExternalInput")
    id8 = nc.dram_tensor("id8", [H, H], F32, kind="ExternalInput")
    id8b = nc.dram_tensor("id8b", [H, H], BF16, kind="ExternalInput")

    enh = nc.dram_tensor("enh", [RP, DIM], F32, kind="ExternalOutput")
    bavg = nc.dram_tensor("bavg", [1, RP], F32, kind="ExternalOutput")

    # columns of the last block holding padded (invalid) rows
    pad_from = valid_rows - (nblk - 1) * BLK
    assert 0 < pad_from <= BLK

    with tile.TileContext(nc) as tc:
        with tc.tile_pool(name="consts", bufs=1) as cp, \
             tc.tile_pool(name="dram", bufs=1, space="DRAM") as dramp:
            ablkt = cp.tile([128, 4 * H], BF16)
            nc.sync.dma_start(
                ablkt[:].rearrange("p (j h) -> p j h", h=H),
                ablk[:].rearrange("(j p) h -> p j h", p=128),
            )
            ch1t = cp.tile([H, 1], F32)
            nc.sync.dma_start(ch1t[:], ch1[:])
            c08t = cp.tile([H, 1], F32)
            nc.sync.dma_start(c08t[:], c08[:])
            ones_bf = cp.tile([TILE, 1], BF16)
            nc.vector.memset(ones_bf[:], 1.0)
            ones8 = cp.tile([H, 1], F32R)
            nc.vector.memset(ones8[:].bitcast(F32), 1.0)

            w1adt = cp.tile([128, 4 * 128], F32R)
            nc.sync.dma_start(
                w1adt[:].rearrange("p (j e) -> p j e", e=128),
                w1ad[:].rearrange("(j p) e -> p j e", p=128),
            )
            w1bdt = cp.tile([128, 4 * 128], F32)
            nc.sync.dma_start(
                w1bdt[:].rearrange("p (j e) -> p j e", e=128),
                w1bd[:].rearrange("(j p) e -> p j e", p=128),
            )
            b1tt = cp.tile([128, 4], F32)
            nc.sync.dma_start(
                b1tt[:], b1t[:].rearrange("(j p) o -> p (j o)", p=128)
            )
            w2t = cp.tile([128, 4 * H], F32R)
            nc.sync.dma_start(
                w2t[:].rearrange("p (j h) -> p j h", h=H),
                w2blk[:].rearrange("(j p) h -> p j h", p=128),
            )
            b2t = cp.tile([H, 1], F32)
            nc.sync.dma_start(b2t[:], b2c[:])
            woutt = cp.tile([128, 4 * DIM], F32)
            nc.sync.dma_start(
                woutt[:].rearrange("p (j m) -> p j m", m=DIM),
                wout[:].rearrange("(j p) m -> p j m", p=128),
            )
            selt = cp.tile([H, DIM], F32R)
            nc.sync.dma_start(selt[:], seld[:])
            woutrt = cp.tile([128, 4 * DIM], F32R)
            nc.sync.dma_start(
                woutrt[:].rearrange("p (j m) -> p j m", m=DIM),
                woutr[:].rearrange("(j p) m -> p j m", p=128),
            )
            mask8t = cp.tile([128, 4 * H], F32)
            nc.sync.dma_start(
                mask8t[:].rearrange("p (j h) -> p j h", h=H),
                mask8d[:].rearrange("(j p) h -> p j h", p=128),
            )
            id8t = cp.tile([H, H], F32)
            nc.sync.dma_start(id8t[:], id8[:])
            id8bt = cp.tile([H, H], BF16)
            nc.sync.dma_start(id8bt[:], id8b[:])

            gwext = cp.tile([H, DIM], F32R)
            c1s = cp.tile([128, 4], F32)
            gtcol = cp.tile([128, 4], F32)
            bavgs = cp.tile([1, RP], F32)

            # ================================================== PHASE A ==
            with tc.tile_pool(name="pa_x", bufs=2) as pax, \
                 tc.tile_pool(name="pa_s", bufs=4) as pas, \
                 tc.tile_pool(name="pa_eps", bufs=3, space="PSUM") as paeps, \
                 tc.tile_pool(name="pa_tps", bufs=2, space="PSUM") as patps, \
                 tc.tile_pool(name="pa_acc", bufs=1, space="PSUM") as paacc:
                poolacc = paacc.tile([H, DIM], F32)
                seacc = paacc.tile([H, 1], F32)
                for s0 in range(0, nblk, ASUPER):
                    cnt = min(ASUPER, nblk - s0)
                    ncols = cnt * BLK
                    s = s0 // ASUPER
                    xn = pax.tile([TILE, ASUPER * 4 * DIM], BF16, tag="xn")
                    nc.sync.dma_start(
                        xn[:, : cnt * 4 * DIM].rearrange(
                            "p (t d) -> p t d", d=DIM
                        ),
                        xbf[:, s, 0 : cnt * 4, :],
                    )
                    xtn = pax.tile([TILE, ASUPER * 4 * BLK], BF16, tag="xtn")
                    nc.sync.dma_start(
                        xtn[:, : cnt * 4 * BLK].rearrange(
                            "p (j n) -> p j n", n=ncols
                        ),
                        xtbf[:, s, :, 0 : cnt * BLK],
                    )
                    for bi in range(cnt):
                        b = s0 + bi
                        et_ps = paeps.tile([H, BLK], F32, tag="etps")
                        for j in range(4):
                            nc.tensor.matmul(
                                et_ps[:],
                                ablkt[:, j * H : (j + 1) * H],
                                xtn[:, j * ncols + bi * BLK :
                                       j * ncols + (bi + 1) * BLK],
                                start=(j == 0), stop=(j == 3),
                            )
                        # m = max(z, 0.2 z - 0.8 c);  p = exp(m + c) = exp(lrelu(z + c))
                        t1 = pas.tile([H, BLK], F32, tag="t1")
                        nc.vector.tensor_scalar(
                            t1[:], et_ps[:], 0.2, c08t[:],
                            ALU.mult, ALU.subtract,
                        )
                        nc.vector.tensor_tensor(t1[:], et_ps[:], t1[:], ALU.max)
                        p1 = pas.tile([H, BLK], BF16, tag="p1")
                        nc.scalar.activation(
                            p1[:], t1[:], AF.Exp, bias=ch1t[:], scale=1.0
                        )
                        if b == nblk - 1 and pad_from < BLK:
                            nc.vector.memset(
                                p1[:, pad_from:BLK].bitcast(F32), 0.0
                            )
                        first, last = b == 0, b == nblk - 1
                        for t in range(4):
                            tr_ps = patps.tile([128, H], BF16, tag="trps")
                            nc.tensor.transpose(
                                tr_ps[:],
                                p1[:, t * 128 : (t + 1) * 128],
                                id8bt[:],
                            )
                            pn = pas.tile([128, H], BF16, tag="pn")
                            if t % 2 == 0:
                                nc.scalar.copy(pn[:], tr_ps[:])
                            else:
                                nc.vector.tensor_copy(pn[:], tr_ps[:])
                            xslice = xn[:, (bi * 4 + t) * DIM :
                                           (bi * 4 + t + 1) * DIM]
                            nc.tensor.matmul(
                                poolacc[:], pn[:], xslice,
                                start=(first and t == 0),
                                stop=(last and t == 3),
                                skip_group_check=True,
                            )
                            nc.tensor.matmul(
                                seacc[:], pn[:], ones_bf[:],
                                start=(first and t == 0),
                                stop=(last and t == 3),
                                skip_group_check=True,
                            )

                pool_s = cp.tile([H, DIM], F32)
                nc.scalar.copy(pool_s[:], poolacc[:])
                se_s = cp.tile([H, 1], F32)
                nc.scalar.copy(se_s[:], seacc[:])

            # ============================================== COLLECTIVE ==
            cc_in = dramp.tile([H, DIM + 1], F32)
            cc_out = dramp.tile([H, DIM + 1], F32)
            nc.sync.dma_start(cc_in[:, 0:DIM], pool_s[:])
            nc.sync.dma_start(cc_in[:, DIM : DIM + 1], se_s[:])
            nc.gpsimd.collective_compute(
                "AllReduce",
                ALU.add,
                replica_groups=[list(range(N_CORES))],
                ins=[cc_in.opt()],
                outs=[cc_out.opt()],
            )
            with tc.tile_pool(name="g_ps", bufs=2, space="PSUM") as gps:
                gsum = cp.tile([H, DIM + 1], F32)
                nc.sync.dma_start(gsum[:], cc_out[:])
                recip = cp.tile([H, 1], F32)
                nc.vector.reciprocal(recip[:], gsum[:, DIM : DIM + 1])
                g8 = cp.tile([H, DIM], F32)
                nc.vector.tensor_scalar(
                    g8[:], gsum[:, 0:DIM], recip[:], None, ALU.mult
                )
                gw_ps = gps.tile([H, DIM], F32, tag="gw")
                for j in range(4):
                    tr_ps = gps.tile([128, H], F32, tag="tr")
                    nc.tensor.transpose(
                        tr_ps[:], g8[:, j * 128 : (j + 1) * 128], id8t[:]
                    )
                    gt_j = cp.tile([128, H], F32, tag=f"gt{j}")
                    nc.scalar.copy(gt_j[:], tr_ps[:])
                    msk_j = mask8t[:, j * H : (j + 1) * H]
                    nc.vector.tensor_tensor(gt_j[:], gt_j[:], msk_j, ALU.mult)
                    nc.vector.tensor_reduce(
                        gtcol[:, j : j + 1], gt_j[:],
                        mybir.AxisListType.X, ALU.add,
                    )
                    gsel_j = cp.tile([128, H], F32, tag=f"gsel{j}")
                    nc.vector.tensor_scalar(
                        gsel_j[:], msk_j, gtcol[:, j : j + 1], None, ALU.mult
                    )
                    nc.tensor.matmul(
                        gw_ps[:], gsel_j[:],
                        woutt[:, j * DIM : (j + 1) * DIM],
                        start=(j == 0), stop=(j == 3),
                    )
                    c1_ps = gps.tile([128, 1], F32, tag="c1ps")
                    nc.tensor.matmul(
                        c1_ps[:], w1bdt[:, j * 128 : (j + 1) * 128],
                        gtcol[:, j : j + 1], start=True, stop=True,
                    )
                    nc.scalar.activation(
                        c1s[:, j : j + 1], c1_ps[:], AF.Identity,
                        bias=b1tt[:, j : j + 1],
                    )
                nc.scalar.copy(gwext[0:H, :], gw_ps[:])

            # ================================================== PHASE B ==
            with tc.tile_pool(name="pb_x", bufs=3) as pbx, \
                 tc.tile_pool(name="pb_w", bufs=3) as pbw, \
                 tc.tile_pool(name="pb_o", bufs=2) as pbo, \
                 tc.tile_pool(name="pb_hps", bufs=2, space="PSUM") as pbhps, \
                 tc.tile_pool(name="pb_bps", bufs=1, space="PSUM") as pbbps, \
                 tc.tile_pool(name="pb_cps", bufs=2, space="PSUM") as pbcps, \
                 tc.tile_pool(name="pb_ops", bufs=2, space="PSUM") as pbops:
                for b in range(nblk):
                    n0 = b * BLK
                    xtr = pbx.tile([128, 4 * BLK], F32R, tag="xtr")
                    nc.sync.dma_start(
                        xtr[:].rearrange("p (j n) -> p j n", n=BLK),
                        xt[:, b, :, :],
                    )
                    beta_ps = pbbps.tile([H, BLK], F32, tag="betaps")
                    for j in range(4):
                        hid_ps = pbhps.tile([128, BLK], F32, tag="hidps")
                        nc.tensor.matmul(
                            hid_ps[:], w1adt[:, j * 128 : (j + 1) * 128],
                            xtr[:, j * BLK : (j + 1) * BLK],
                            start=True, stop=True,
                        )
                        hid_s = pbw.tile([128, BLK], F32R, tag="hids")
                        nc.scalar.activation(
                            hid_s[:], hid_ps[:], AF.Relu,
                            bias=c1s[:, j : j + 1],
                        )
                        nc.tensor.matmul(
                            beta_ps[:], w2t[:, j * H : (j + 1) * H], hid_s[:],
                            start=(j == 0), stop=(j == 3),
                        )
                    betax = pbw.tile([H, BLK], F32R, tag="betax")
                    nc.scalar.activation(
                        betax[0:H, :], beta_ps[:], AF.Sigmoid, bias=b2t[:]
                    )
                    beta1 = pbw.tile([H, BLK], F32R, tag="beta1")
                    nc.vector.tensor_scalar(
                        beta1[:], betax[0:H, :].bitcast(F32), -1.0, 1.0,
                        ALU.mult, ALU.add,
                    )
                    enh1 = pbo.tile([128, 4 * BLK], F32R, tag="enh1")
                    for j in range(4):
                        bc_ps = pbcps.tile([128, BLK], F32, tag="bcps")
                        nc.tensor.matmul(
                            bc_ps[:], selt[:, j * 128 : (j + 1) * 128],
                            beta1[:], start=True, stop=True,
                        )
                        nc.vector.tensor_tensor(
                            enh1[:, j * BLK : (j + 1) * BLK],
                            xtr[:, j * BLK : (j + 1) * BLK].bitcast(F32),
                            bc_ps[:], ALU.mult,
                        )
                    outb = pbo.tile([128, 4 * DIM], F32, tag="outb")
                    for t in range(4):
                        out_ps = pbops.tile([128, DIM], F32, tag="outps")
                        for j in range(4):
                            nc.tensor.matmul(
                                out_ps[:],
                                enh1[:, j * BLK + t * 128 :
                                        j * BLK + (t + 1) * 128],
                                woutrt[:, j * DIM : (j + 1) * DIM],
                                start=(j == 0), stop=False,
                                skip_group_check=True,
                            )
                        nc.tensor.matmul(
                            out_ps[:], betax[:, t * 128 : (t + 1) * 128],
                            gwext[:], start=False, stop=True,
                            skip_group_check=True,
                        )
                        if t % 2 == 0:
                            nc.vector.tensor_copy(
                                outb[:, t * DIM : (t + 1) * DIM], out_ps[:]
                            )
                        else:
                            nc.scalar.copy(
                                outb[:, t * DIM : (t + 1) * DIM], out_ps[:]
                            )
                    avg_ps = pbbps.tile([1, BLK], F32, tag="avgps")
                    nc.tensor.matmul(
                        avg_ps[:], ones8[:], betax[0:H, :],
                        start=True, stop=True,
                    )
                    nc.scalar.activation(
                        bavgs[:, n0 : n0 + BLK], avg_ps[:], AF.Copy,
                        scale=0.125,
                    )
                    nc.sync.dma_start(
                        enh[n0 : n0 + BLK, :].rearrange(
                            "(t p) d -> p t d", p=128
                        ),
                        outb[:].rearrange("p (t d) -> p t d", d=DIM),
                    )
                nc.sync.dma_start(bavg[:], bavgs[:])
    nc.compile()
    return nc


def _preprocess(entity_embeds, global_memories, attn_vec, W1, b1, W2, b2,
                Wout, bout, nblk, rows_valid):
    RP = nblk * BLK
    ent = np.ascontiguousarray(entity_embeds, dtype=np.float32)
    n = ent.shape[0]
    assert n == N_CORES * rows_valid

    a_g = attn_vec[:, :HD].astype(np.float32)
    a_h = attn_vec[:, HD:].astype(np.float32)
    ablk = np.zeros((DIM, H), np.float32)
    for h in range(H):
        ablk[h * HD : (h + 1) * HD, h] = a_h[h]
    ch = (global_memories.astype(np.float32) * a_g).sum(axis=1).reshape(H, 1)

    W1f = np.asarray(W1, np.float32)
    w1ad = np.zeros((DIM, 128), np.float32)
    w1bd = np.zeros((DIM, 128), np.float32)
    for h in range(H):
        r0 = h * HD
        c0 = (h * HD) % 128
        w1ad[r0 : r0 + HD, c0 : c0 + HD] = W1f[h, :HD, :]
        w1bd[r0 : r0 + HD, c0 : c0 + HD] = W1f[h, HD:, :]
    b1t = np.asarray(b1, np.float32).reshape(DIM, 1)
    w2blk = np.zeros((DIM, H), np.float32)
    for h in range(H):
        w2blk[h * HD : (h + 1) * HD, h] = np.asarray(W2[h], np.float32)
    b2c = np.asarray(b2, np.float32).reshape(H, 1)

    heads = np.arange(DIM) // HD
    sel = (heads[None, :] == np.arange(H)[:, None]).astype(np.float32)
    mask8 = sel.T.copy()
    id8 = np.eye(H, dtype=np.float32)

    common = {
        "ablk": ablk.astype(ml_dtypes.bfloat16),
        "ch1": ch,
        "c08": 0.8 * ch,
        "w1ad": w1ad,
        "w1bd": w1bd,
        "b1t": b1t,
        "w2blk": w2blk,
        "b2c": b2c,
        "wout": np.asarray(Wout, np.float32),
        "woutr": np.asarray(Wout, np.float32),
        "seld": np.ascontiguousarray(sel),
        "mask8d": np.ascontiguousarray(mask8),
        "id8": id8,
        "id8b": id8.astype(ml_dtypes.bfloat16),
    }
    ns = (nblk + ASUPER - 1) // ASUPER
    RP2 = ns * ASUPER * BLK
    in_maps = []
    for c in range(N_CORES):
        sl = ent[c * rows_valid : (c + 1) * rows_valid]
        pad = np.zeros((RP2, DIM), np.float32)
        pad[:rows_valid] = sl
        padb = pad.astype(ml_dtypes.bfloat16)
        # natural swizzle: [p, s, t, d] = pad[s*2048 + t*128 + p, d]
        xbf_sw = np.ascontiguousarray(
            padb.reshape(ns, 4 * ASUPER, 128, DIM).transpose(2, 0, 1, 3)
        )
        # transposed swizzles from pad.T (DIM, RP2)
        xtb = np.ascontiguousarray(pad.T).astype(ml_dtypes.bfloat16)
        xtbf_sw = np.ascontiguousarray(
            xtb.reshape(4, 128, ns, ASUPER * BLK).transpose(1, 2, 0, 3)
        )
        xtf = np.ascontiguousarray(pad[:RP].T)  # (DIM, RP) fp32
        xt_sw = np.ascontiguousarray(
            xtf.reshape(4, 128, nblk, BLK).transpose(1, 2, 0, 3)
        )
        m = dict(common)
        m["xbf"] = xbf_sw
        m["xtbf"] = xtbf_sw
        m["xt"] = xt_sw
        in_maps.append(m)
    return in_maps


def run(entity_embeds, global_memories, attn_vec, W1, b1, W2, b2, Wout, bout,
        nblk=None, trace=False):
    n = entity_embeds.shape[0]
    rows = n // N_CORES
    if nblk is None:
        nblk = (rows + BLK - 1) // BLK
    key = (nblk, rows)
    if key not in _cache:
        _cache[key] = _build(nblk, rows)
    nc = _cache[key]
    in_maps = _preprocess(
        entity_embeds, global_memories, attn_vec, W1, b1, W2, b2, Wout, bout,
        nblk, rows,
    )
    res = run_bass_kernel_spmd(
        nc, in_maps, core_ids=list(range(N_CORES)), trace=trace
    )
    enh = np.concatenate(
        [res.results[c]["enh"][:rows] for c in range(N_CORES)], axis=0
    )
    enh += np.asarray(bout, np.float32)[None, :]
    bavg = np.concatenate(
        [res.results[c]["bavg"][0, :rows] for c in range(N_CORES)], axis=0
    )
    return (enh, bavg), res


def kernel(entity_embeds, global_memories, attn_vec, W1, b1, W2, b2, Wout,
           bout):
    (enh, bavg), _ = run(
        entity_embeds, global_memories, attn_vec, W1, b1, W2, b2, Wout, bout
    )
    return enh, bavg


# revision 15
# speedup vs baseline: 1.1232x; 1.0659x over previous
"""Trainium2 Bass kernel for nn_MultiHeadGlobalMemory.

Contract: kernel(**inputs) takes FULL unsharded numpy inputs and returns the
FULL outputs (enhanced (N, DIM) fp32, beta_avg (N,) fp32), matching
reference.reference(). Shards entity rows over 8 NeuronCores; two-phase
kernel with one small AllReduce between phases.

Phase A (softmax stats, bf16): per 512-row block, e^T = Ablk^T @ X^T on PE,
p = max(exp(z), exp(0.2 z)) (== exp(leaky_relu(z))) on ACT/DVE, transpose p
to natural layout, then pool += p^T @ X and sumexp += p^T @ 1 accumulate in
PSUM across all blocks.  AllReduce(pool, sumexp) over the 8 cores.

Phase B (fp32): g = pool/sumexp, c1 = W1b^T g + b1, Gw[h,:] = per-head
g-weighted rows of Wout (+ bout row).  Per block: hidT = relu(W1a^T X^T +
c1), betaT = sigmoid(W2^T hidT + b2), bcast (1-beta) over head partitions
via sel-matmul, enh1T = X^T * (1-B)^T, out = enh1^T^T @ Wout + beta_ext @
Gw_ext (fused bias), beta_avg = mean(beta).

Softmax runs without max-subtraction (e is O(5), exp safe in fp32); the
pool/sumexp ratio is mathematically identical to the reference softmax.
"""

import sys
import types

import numpy as np

# ---------------------------------------------------------------- axon hook
def _install_ntff_hook():
    """Provide antenv.axon_hooks so trace=True works under axon (optional)."""
    if "antenv.axon_hooks" in sys.modules:
        return
    try:
        state = {"hook": None}
        mod = types.ModuleType("antenv.axon_hooks")
        mod.set_axon_ntff_profile_hook = lambda h: state.__setitem__("hook", h)
        mod.get_axon_ntff_profile_hook = lambda: state["hook"]
        sys.modules["antenv.axon_hooks"] = mod
        from trn_agent_boot.trn_boot import _ntff_profile_via_ctypes

        mod.set_axon_ntff_profile_hook(
            _ntff_profile_via_ctypes("/opt/axon/libaxon_pjrt.so")
        )
    except Exception:
        pass


_install_ntff_hook()

import ml_dtypes  # noqa: E402
import concourse.bacc as bacc  # noqa: E402
import concourse.tile as tile  # noqa: E402
from concourse import mybir  # noqa: E402
from concourse.bass_utils import run_bass_kernel_spmd  # noqa: E402

F32 = mybir.dt.float32
F32R = mybir.dt.float32r
BF16 = mybir.dt.bfloat16
AF = mybir.ActivationFunctionType
ALU = mybir.AluOpType

N_CORES = 8
DIM, H, HD = 512, 8, 64
BLK = 512
TILE = 128
ASUPER = 4          # phase-A blocks per DMA superblock

_cache = {}


def _build(nblk: int, valid_rows: int):
    RP = nblk * BLK
    nc = bacc.Bacc("TRN2", target_bir_lowering=False, num_devices=N_CORES)

    ns = (nblk + ASUPER - 1) // ASUPER
    xbf = nc.dram_tensor("xbf", [128, ns, 4 * ASUPER, DIM], BF16, kind="ExternalInput")
    xtbf = nc.dram_tensor("xtbf", [128, ns, 4, ASUPER * BLK], BF16, kind="ExternalInput")
    xt = nc.dram_tensor("xt", [128, nblk, 4, BLK], F32R, kind="ExternalInput")
    ablk = nc.dram_tensor("ablk", [DIM, H], BF16, kind="ExternalInput")
    ch1 = nc.dram_tensor("ch1", [H, 1], F32, kind="ExternalInput")
    c08 = nc.dram_tensor("c08", [H, 1], F32, kind="ExternalInput")
    w1ad = nc.dram_tensor("w1ad", [DIM, 128], F32R, kind="ExternalInput")
    w1bd = nc.dram_tensor("w1bd", [DIM, 128], F32, kind="ExternalInput")
    b1t = nc.dram_tensor("b1t", [DIM, 1], F32, kind="ExternalInput")
    w2blk = nc.dram_tensor("w2blk", [DIM, H], F32R, kind="ExternalInput")
    b2c = nc.dram_tensor("b2c", [H, 1], F32, kind="ExternalInput")
    wout = nc.dram_tensor("wout", [DIM, DIM], F32, kind="ExternalInput")
    woutr = nc.dram_tensor("woutr", [DIM, DIM], F32R, kind="ExternalInput")
    seld = nc.dram_tensor("seld", [H, DIM], F32R, kind="ExternalInput")
    mask8d = nc.dram_tensor("mask8d", [DIM, H], F32, kind="ExternalInput")
    id8 = nc.dram_tensor("id8", [H, H], F32, kind="ExternalInput")
    id8b = nc.dram_tensor("id8b", [H, H], BF16, kind="ExternalInput")

    enh = nc.dram_tensor("enh", [RP, DIM], F32, kind="ExternalOutput")
    bavg = nc.dram_tensor("bavg", [1, RP], F32, kind="ExternalOutput")

    # columns of the last block holding padded (invalid) rows
    pad_from = valid_rows - (nblk - 1) * BLK
    assert 0 < pad_from <= BLK

    with tile.TileContext(nc) as tc:
        with tc.tile_pool(name="consts", bufs=1) as cp, \
             tc.tile_pool(name="dram", bufs=1, space="DRAM") as dramp:
            ablkt = cp.tile([128, 4 * H], BF16)
            nc.sync.dma_start(
                ablkt[:].rearrange("p (j h) -> p j h", h=H),
                ablk[:].rearrange("(j p) h -> p j h", p=128),
            )
            ch1t = cp.tile([H, 1], F32)
            nc.sync.dma_start(ch1t[:], ch1[:])
            c08t = cp.tile([H, 1], F32)
            nc.sync.dma_start(c08t[:], c08[:])
            ones_bf = cp.tile([TILE, 1], BF16)
            nc.vector.memset(ones_bf[:], 1.0)
            ones8 = cp.tile([H, 1], F32R)
            nc.vector.memset(ones8[:].bitcast(F32), 1.0)

            w1adt = cp.tile([128, 4 * 128], F32R)
            nc.sync.dma_start(
                w1adt[:].rearrange("p (j e) -> p j e", e=128),
                w1ad[:].rearrange("(j p) e -> p j e", p=128),
            )
            w1bdt = cp.tile([128, 4 * 128], F32)
            nc.sync.dma_start(
                w1bdt[:].rearrange("p (j e) -> p j e", e=128),
                w1bd[:].rearrange("(j p) e -> p j e", p=128),
            )
            b1tt = cp.tile([128, 4], F32)
            nc.sync.dma_start(
                b1tt[:], b1t[:].rearrange("(j p) o -> p (j o)", p=128)
            )
            w2t = cp.tile([128, 4 * H], F32R)
            nc.sync.dma_start(
                w2t[:].rearrange("p (j h) -> p j h", h=H),
                w2blk[:].rearrange("(j p) h -> p j h", p=128),
            )
            b2t = cp.tile([H, 1], F32)
            nc.sync.dma_start(b2t[:], b2c[:])
            woutt = cp.tile([128, 4 * DIM], F32)
            nc.sync.dma_start(
                woutt[:].rearrange("p (j m) -> p j m", m=DIM),
                wout[:].rearrange("(j p) m -> p j m", p=128),
            )
            selt = cp.tile([H, DIM], F32R)
            nc.sync.dma_start(selt[:], seld[:])
            woutrt = cp.tile([128, 4 * DIM], F32R)
            nc.sync.dma_start(
                woutrt[:].rearrange("p (j m) -> p j m", m=DIM),
                woutr[:].rearrange("(j p) m -> p j m", p=128),
            )
            mask8t = cp.tile([128, 4 * H], F32)
            nc.sync.dma_start(
                mask8t[:].rearrange("p (j h) -> p j h", h=H),
                mask8d[:].rearrange("(j p) h -> p j h", p=128),
            )
            id8t = cp.tile([H, H], F32)
            nc.sync.dma_start(id8t[:], id8[:])
            id8bt = cp.tile([H, H], BF16)
            nc.sync.dma_start(id8bt[:], id8b[:])

            gwext = cp.tile([H, DIM], F32R)
            c1s = cp.tile([128, 4], F32)
            gtcol = cp.tile([128, 4], F32)
            bavgs = cp.tile([1, RP], F32)

            # ================================================== PHASE A ==
            with tc.tile_pool(name="pa_x", bufs=2) as pax, \
                 tc.tile_pool(name="pa_s", bufs=4) as pas, \
                 tc.tile_pool(name="pa_eps", bufs=3, space="PSUM") as paeps, \
                 tc.tile_pool(name="pa_tps", bufs=2, space="PSUM") as patps, \
                 tc.tile_pool(name="pa_acc", bufs=1, space="PSUM") as paacc:
                poolacc = paacc.tile([H, DIM], F32)
                seacc = paacc.tile([H, 1], F32)
                for s0 in range(0, nblk, ASUPER):
                    cnt = min(ASUPER, nblk - s0)
                    ncols = cnt * BLK
                    s = s0 // ASUPER
                    xn = pax.tile([TILE, ASUPER * 4 * DIM], BF16, tag="xn")
                    nc.sync.dma_start(
                        xn[:, : cnt * 4 * DIM].rearrange(
                            "p (t d) -> p t d", d=DIM
                        ),
                        xbf[:, s, 0 : cnt * 4, :],
                    )
                    xtn = pax.tile([TILE, ASUPER * 4 * BLK], BF16, tag="xtn")
                    nc.sync.dma_start(
                        xtn[:, : cnt * 4 * BLK].rearrange(
                            "p (j n) -> p j n", n=ncols
                        ),
                        xtbf[:, s, :, 0 : cnt * BLK],
                    )
                    for bi in range(cnt):
                        b = s0 + bi
                        et_ps = paeps.tile([H, BLK], F32, tag="etps")
                        for j in range(4):
                            nc.tensor.matmul(
                                et_ps[:],
                                ablkt[:, j * H : (j + 1) * H],
                                xtn[:, j * ncols + bi * BLK :
                                       j * ncols + (bi + 1) * BLK],
                                start=(j == 0), stop=(j == 3),
                            )
                        # m = max(z, 0.2 z - 0.8 c);  p = exp(m + c) = exp(lrelu(z + c))
                        t1 = pas.tile([H, BLK], F32, tag="t1")
                        nc.vector.tensor_scalar(
                            t1[:], et_ps[:], 0.2, c08t[:],
                            ALU.mult, ALU.subtract,
                        )
                        nc.vector.tensor_tensor(t1[:], et_ps[:], t1[:], ALU.max)
                        p1 = pas.tile([H, BLK], BF16, tag="p1")
                        nc.scalar.activation(
                            p1[:], t1[:], AF.Exp, bias=ch1t[:], scale=1.0
                        )
                        if b == nblk - 1 and pad_from < BLK:
                            nc.vector.memset(
                                p1[:, pad_from:BLK].bitcast(F32), 0.0
                            )
                        first, last = b == 0, b == nblk - 1
                        for t in range(4):
                            tr_ps = patps.tile([128, H], BF16, tag="trps")
                            nc.tensor.transpose(
                                tr_ps[:],
                                p1[:, t * 128 : (t + 1) * 128],
                                id8bt[:],
                            )
                            pn = pas.tile([128, H], BF16, tag="pn")
                            if t % 2 == 0:
                                nc.scalar.copy(pn[:], tr_ps[:])
                            else:
                                nc.vector.tensor_copy(pn[:], tr_ps[:])
                            xslice = xn[:, (bi * 4 + t) * DIM :
                                           (bi * 4 + t + 1) * DIM]
                            nc.tensor.matmul(
                                poolacc[:], pn[:], xslice,
                                start=(first and t == 0),
                                stop=(last and t == 3),
                                skip_group_check=True,
                            )
                            nc.tensor.matmul(
                                seacc[:], pn[:], ones_bf[:],
                                start=(first and t == 0),
                                stop=(last and t == 3),
                                skip_group_check=True,
                            )

                pool_s = cp.tile([H, DIM], F32)
                nc.scalar.copy(pool_s[:], poolacc[:])
                se_s = cp.tile([H, 1], F32)
                nc.scalar.copy(se_s[:], seacc[:])

            # ============================================== COLLECTIVE ==
            cc_in = dramp.tile([H, DIM + 1], F32)
            cc_out = dramp.tile([H, DIM + 1], F32)
            nc.sync.dma_start(cc_in[:, 0:DIM], pool_s[:])
            nc.sync.dma_start(cc_in[:, DIM : DIM + 1], se_s[:])
            nc.gpsimd.collective_compute(
                "AllReduce",
                ALU.add,
                replica_groups=[list(range(N_CORES))],
                ins=[cc_in.opt()],
                outs=[cc_out.opt()],
            )
            with tc.tile_pool(name="g_ps", bufs=2, space="PSUM") as gps:
                gsum = cp.tile([H, DIM + 1], F32)
                nc.sync.dma_start(gsum[:], cc_out[:])
                recip = cp.tile([H, 1], F32)
                nc.vector.reciprocal(recip[:], gsum[:, DIM : DIM + 1])
                g8 = cp.tile([H, DIM], F32)
                nc.vector.tensor_scalar(
                    g8[:], gsum[:, 0:DIM], recip[:], None, ALU.mult
                )
                gw_ps = gps.tile([H, DIM], F32, tag="gw")
                for j in range(4):
                    tr_ps = gps.tile([128, H], F32, tag="tr")
                    nc.tensor.transpose(
                        tr_ps[:], g8[:, j * 128 : (j + 1) * 128], id8t[:]
                    )
                    gt_j = cp.tile([128, H], F32, tag=f"gt{j}")
                    nc.scalar.copy(gt_j[:], tr_ps[:])
                    msk_j = mask8t[:, j * H : (j + 1) * H]
                    nc.vector.tensor_tensor(gt_j[:], gt_j[:], msk_j, ALU.mult)
                    nc.vector.tensor_reduce(
                        gtcol[:, j : j + 1], gt_j[:],
                        mybir.AxisListType.X, ALU.add,
                    )
                    gsel_j = cp.tile([128, H], F32, tag=f"gsel{j}")
                    nc.vector.tensor_scalar(
                        gsel_j[:], msk_j, gtcol[:, j : j + 1], None, ALU.mult
                    )
                    nc.tensor.matmul(
                        gw_ps[:], gsel_j[:],
                        woutt[:, j * DIM : (j + 1) * DIM],
                        start=(j == 0), stop=(j == 3),
                    )
                    c1_ps = gps.tile([128, 1], F32, tag="c1ps")
                    nc.tensor.matmul(
                        c1_ps[:], w1bdt[:, j * 128 : (j + 1) * 128],
                        gtcol[:, j : j + 1], start=True, stop=True,
                    )
                    nc.scalar.activation(
                        c1s[:, j : j + 1], c1_ps[:], AF.Identity,
                        bias=b1tt[:, j : j + 1],
                    )
                nc.scalar.copy(gwext[0:H, :], gw_ps[:])

            # ================================================== PHASE B ==
            with tc.tile_pool(name="pb_x", bufs=3) as pbx, \
                 tc.tile_pool(name="pb_w", bufs=3) as pbw, \
                 tc.tile_pool(name="pb_o", bufs=2) as pbo, \
                 tc.tile_pool(name="pb_hps", bufs=2, space="PSUM") as pbhps, \
                 tc.tile_pool(name="pb_bps", bufs=1, space="PSUM") as pbbps, \
                 tc.tile_pool(name="pb_cps", bufs=2, space="PSUM") as pbcps, \
                 tc.tile_pool(name="pb_ops", bufs=2, space="PSUM") as pbops:
                for b in range(nblk):
                    n0 = b * BLK
                    xtr = pbx.tile([128, 4 * BLK], F32R, tag="xtr")
                    nc.sync.dma_start(
                        xtr[:].rearrange("p (j n) -> p j n", n=BLK),
                        xt[:, b, :, :],
                    )
                    beta_ps = pbbps.tile([H, BLK], F32, tag="betaps")
                    for j in range(4):
                        hid_ps = pbhps.tile([128, BLK], F32, tag="hidps")
                        nc.tensor.matmul(
                            hid_ps[:], w1adt[:, j * 128 : (j + 1) * 128],
                            xtr[:, j * BLK : (j + 1) * BLK],
                            start=True, stop=True,
                        )
                        hid_s = pbw.tile([128, BLK], F32R, tag="hids")
                        nc.scalar.activation(
                            hid_s[:], hid_ps[:], AF.Relu,
                            bias=c1s[:, j : j + 1],
                        )
                        nc.tensor.matmul(
                            beta_ps[:], w2t[:, j * H : (j + 1) * H], hid_s[:],
                            start=(j == 0), stop=(j == 3),
                        )
                    betax = pbw.tile([H, BLK], F32R, tag="betax")
                    nc.scalar.activation(
                        betax[0:H, :], beta_ps[:], AF.Sigmoid, bias=b2t[:]
                    )
                    beta1 = pbw.tile([H, BLK], F32R, tag="beta1")
                    nc.vector.tensor_scalar(
                        beta1[:], betax[0:H, :].bitcast(F32), -1.0, 1.0,
                        ALU.mult, ALU.add,
                    )
                    enh1 = pbo.tile([128, 4 * BLK], F32R, tag="enh1")
                    for j in range(4):
                        bc_ps = pbcps.tile([128, BLK], F32, tag="bcps")
                        nc.tensor.matmul(
                            bc_ps[:], selt[:, j * 128 : (j + 1) * 128],
                            beta1[:], start=True, stop=True,
                        )
                        nc.vector.tensor_tensor(
                            enh1[:, j * BLK : (j + 1) * BLK],
                            xtr[:, j * BLK : (j + 1) * BLK].bitcast(F32),
                            bc_ps[:], ALU.mult,
                        )
                    outb = pbo.tile([128, 4 * DIM], F32, tag="outb")
                    for t in range(4):
                        out_ps = pbops.tile([128, DIM], F32, tag="outps")
                        for j in range(4):
                            nc.tensor.matmul(
                                out_ps[:],
                                enh1[:, j * BLK + t * 128 :
                                        j * BLK + (t + 1) * 128],
                                woutrt[:, j * DIM : (j + 1) * DIM],
                                start=(j == 0), stop=False,
                                skip_group_check=True,
                            )
                        nc.tensor.matmul(
                            out_ps[:], betax[:, t * 128 : (t + 1) * 128],
                            gwext[:], start=False, stop=True,
                            skip_group_check=True,
                        )
                        nc.vector.tensor_copy(
                            outb[:, t * DIM : (t + 1) * DIM], out_ps[:]
                        )
                    avg_ps = pbbps.tile([1, BLK], F32, tag="avgps")
                    nc.tensor.matmul(
                        avg_ps[:], ones8[:], betax[0:H, :],
                        start=True, stop=True,
                    )
                    nc.scalar.activation(
                        bavgs[:, n0 : n0 + BLK], avg_ps[:], AF.Copy,
                        scale=0.125,
                    )
                    nc.sync.dma_start(
                        enh[n0 : n0 + BLK, :].rearrange(
                            "(t p) d -> p t d", p=128
                        ),
                        outb[:].rearrange("p (t d) -> p t d", d=DIM),
                    )
                nc.sync.dma_start(bavg[:], bavgs[:])
    nc.compile()
    return nc


def _preprocess(entity_embeds, global_memories, attn_vec, W1, b1, W2, b2,
                Wout, bout, nblk, rows_valid):
    RP = nblk * BLK
    ent = np.ascontiguousarray(entity_embeds, dtype=np.float32)
    n = ent.shape[0]
    assert n == N_CORES * rows_valid

    a_g = attn_vec[:, :HD].astype(np.float32)
    a_h = attn_vec[:, HD:].astype(np.float32)
    ablk = np.zeros((DIM, H), np.float32)
    for h in range(H):
        ablk[h * HD : (h + 1) * HD, h] = a_h[h]
    ch = (global_memories.astype(np.float32) * a_g).sum(axis=1).reshape(H, 1)

    W1f = np.asarray(W1, np.float32)
    w1ad = np.zeros((DIM, 128), np.float32)
    w1bd = np.zeros((DIM, 128), np.float32)
    for h in range(H):
        r0 = h * HD
        c0 = (h * HD) % 128
        w1ad[r0 : r0 + HD, c0 : c0 + HD] = W1f[h, :HD, :]
        w1bd[r0 : r0 + HD, c0 : c0 + HD] = W1f[h, HD:, :]
    b1t = np.asarray(b1, np.float32).reshape(DIM, 1)
    w2blk = np.zeros((DIM, H), np.float32)
    for h in range(H):
        w2blk[h * HD : (h + 1) * HD, h] = np.asarray(W2[h], np.float32)
    b2c = np.asarray(b2, np.float32).reshape(H, 1)

    heads = np.arange(DIM) // HD
    sel = (heads[None, :] == np.arange(H)[:, None]).astype(np.float32)
    mask8 = sel.T.copy()
    id8 = np.eye(H, dtype=np.float32)

    common = {
        "ablk": ablk.astype(ml_dtypes.bfloat16),
        "ch1": ch,
        "c08": 0.8 * ch,
        "w1ad": w1ad,
        "w1bd": w1bd,
        "b1t": b1t,
        "w2blk": w2blk,
        "b2c": b2c,
        "wout": np.asarray(Wout, np.float32),
        "woutr": np.asarray(Wout, np.float32),
        "seld": np.ascontiguousarray(sel),
        "mask8d": np.ascontiguousarray(mask8),
        "id8": id8,
        "id8b": id8.astype(ml_dtypes.bfloat16),
    }
    ns = (nblk + ASUPER - 1) // ASUPER
    RP2 = ns * ASUPER * BLK
    in_maps = []
    for c in range(N_CORES):
        sl = ent[c * rows_valid : (c + 1) * rows_valid]
        pad = np.zeros((RP2, DIM), np.float32)
        pad[:rows_valid] = sl
        padb = pad.astype(ml_dtypes.bfloat16)
        # natural swizzle: [p, s, t, d] = pad[s*2048 + t*128 + p, d]
        xbf_sw = np.ascontiguousarray(
            padb.reshape(ns, 4 * ASUPER, 128, DIM).transpose(2, 0, 1, 3)
        )
        # transposed swizzles from pad.T (DIM, RP2)
        xtb = np.ascontiguousarray(pad.T).astype(ml_dtypes.bfloat16)
        xtbf_sw = np.ascontiguousarray(
            xtb.reshape(4, 128, ns, ASUPER * BLK).transpose(1, 2, 0, 3)
        )
        xtf = np.ascontiguousarray(pad[:RP].T)  # (DIM, RP) fp32
        xt_sw = np.ascontiguousarray(
            xtf.reshape(4, 128, nblk, BLK).transpose(1, 2, 0, 3)
        )
        m = dict(common)
        m["xbf"] = xbf_sw
        m["xtbf"] = xtbf_sw
        m["xt"] = xt_sw
        in_maps.append(m)
    return in_maps


def run(entity_embeds, global_memories, attn_vec, W1, b1, W2, b2, Wout, bout,
        nblk=None, trace=False):
    n = entity_embeds.shape[0]
    rows = n // N_CORES
    if nblk is None:
        nblk = (rows + BLK - 1) // BLK
    key = (nblk, rows)
    if key not in _cache:
        _cache[key] = _build(nblk, rows)
    nc = _cache[key]
    in_maps = _preprocess(
        entity_embeds, global_memories, attn_vec, W1, b1, W2, b2, Wout, bout,
        nblk, rows,
    )
    res = run_bass_kernel_spmd(
        nc, in_maps, core_ids=list(range(N_CORES)), trace=trace
    )
    enh = np.concatenate(
        [res.results[c]["enh"][:rows] for c in range(N_CORES)], axis=0
    )
    enh += np.asarray(bout, np.float32)[None, :]
    bavg = np.concatenate(
        [res.results[c]["bavg"][0, :rows] for c in range(N_CORES)], axis=0
    )
    return (enh, bavg), res


def kernel(entity_embeds, global_memories, attn_vec, W1, b1, W2, b2, Wout,
           bout):
    (enh, bavg), _ = run(
        entity_embeds, global_memories, attn_vec, W1, b1, W2, b2, Wout, bout
    )
    return enh, bavg
